# revision 15
# baseline (speedup 1.0000x reference)
"""Trainium2 Bass kernel for nn_EnsembleModel (ensemble MLP, E=10).

Computes, for each ensemble member e:
    h1 = silu(x @ W1[e] + b1[e])      # [B, 256]
    h2 = silu(h1 @ W2[e] + b2[e])     # [B, 256]
    pred = h2 @ W3[e] + b3[e]         # [B, 48]
    means, logvars = pred[:, :24], pred[:, 24:]

Sharding: data-parallel over the batch axis across 8 NeuronCores; all
ensemble weights are replicated and SBUF-resident on every core.

On-chip layout: features live on SBUF partitions, batch on the free dim.
Matmuls are W.T @ x with the weight stationary ([K, M]) and the batch
streaming as the moving operand (N=512 per PSUM bank).  Silu+bias is fused
into one ScalarE activation per [128, 1024] tile reading PSUM directly.
"""

import numpy as np
import ml_dtypes

import concourse.bacc as bacc
import concourse.mybir as mybir
import concourse.tile as tile
from concourse.bass_utils import run_bass_kernel_spmd

# problem dims (hardcoded per harness contract)
E = 10          # ensemble members
IN = 32         # input dim
H = 256         # hidden
OUT = 24        # output dim (head emits 2*OUT)
OUT2 = 2 * OUT  # 48
B = 65536
NCORES = 8
BL = B // NCORES    # 8192 batch rows per core
BT = 512            # matmul moving free dim (one fp32 PSUM bank)
GRP = 2             # batch tiles per iteration group
GW = BT * GRP       # 1024
NGRP = BL // GW     # 8

F32 = mybir.dt.float32

# compute dtype for matmul operands ("bf16" or "f32")
COMPUTE_DTYPE = "bf16"

# best-known kernel structure (chosen via TimelineSim + HW timing sweeps)
BEST_KW = dict(grp=3, hbufs=2, pw=512, obufs=2, ilv=2)


def build_nc(
    compute="bf16",
    inner_reps=1,
    grp=GRP,
    hbufs=3,
    pred_in_h=False,
    pw=None,
    obufs=1,
    ilv=1,
):
    """Build the per-core Bass program (SPMD: same NEFF on all 8 cores)."""
    dt = mybir.dt.bfloat16 if compute == "bf16" else mybir.dt.float32
    nc = bacc.Bacc("TRN2", target_bir_lowering=False, debug=False)

    # batch groups per member: list of (col offset, width); width = grp*BT
    # except a possibly smaller tail group
    groups = []
    off = 0
    while off < BL:
        w = min(grp * BT, BL - off)
        groups.append((off, w))
        off += w

    xT_d = nc.dram_tensor("xT", [IN, BL], dt, kind="ExternalInput")
    w1_d = nc.dram_tensor("w1", [IN, E * H], dt, kind="ExternalInput")
    w2_d = nc.dram_tensor("w2", [128, E * 2 * H], dt, kind="ExternalInput")
    w3_d = nc.dram_tensor("w3", [128, E * 2 * OUT2], dt, kind="ExternalInput")
    bs_d = nc.dram_tensor("bs", [128, 5 * E], F32, kind="ExternalInput")
    out_d = nc.dram_tensor("out", [E, OUT2, BL], F32, kind="ExternalOutput")

    xT = xT_d.ap()
    w1 = w1_d.ap()
    w2 = w2_d.ap()
    w3 = w3_d.ap()
    bs = bs_d.ap()
    out = out_d.ap()

    Silu = mybir.ActivationFunctionType.Silu

    GWm = grp * BT  # max group width
    if pw is None:
        pw = GWm  # pred-psum chunk width
    assert pw % BT == 0

    with tile.TileContext(nc) as tc:
        with (
            tc.tile_pool(name="consts", bufs=1) as cpool,
            tc.tile_pool(name="hsb", bufs=6) as hpool,
            tc.tile_pool(name="osb", bufs=2) as opool,
            tc.tile_pool(name="hps", bufs=hbufs, space="PSUM") as hps,
            tc.tile_pool(name="ops", bufs=obufs, space="PSUM") as ops,
        ):
            pred_pool = hps if pred_in_h else ops
            xsb = cpool.tile([IN, BL], dt, name="xsb")
            w1sb = cpool.tile([IN, E * H], dt, name="w1sb")
            w2sb = cpool.tile([128, E * 2 * H], dt, name="w2sb")
            w3sb = cpool.tile([128, E * 2 * OUT2], dt, name="w3sb")
            bsb = cpool.tile([128, 5 * E], F32, name="bsb")
            # loads ordered / chunked so the first (g0, e0) iteration's
            # dependencies arrive first and compute can start early
            nc.sync.dma_start(bsb[:], bs)
            c0, gw0 = groups[0]
            nc.sync.dma_start(xsb[:, c0 : c0 + gw0], xT[:, c0 : c0 + gw0])
            for e in range(E):
                nc.sync.dma_start(
                    w1sb[:, e * H : (e + 1) * H], w1[:, e * H : (e + 1) * H]
                )
                nc.sync.dma_start(
                    w2sb[:, e * 2 * H : (e + 1) * 2 * H],
                    w2[:, e * 2 * H : (e + 1) * 2 * H],
                )
                nc.sync.dma_start(
                    w3sb[:, e * 2 * OUT2 : (e + 1) * 2 * OUT2],
                    w3[:, e * 2 * OUT2 : (e + 1) * 2 * OUT2],
                )
            for c0g, gwg in groups[1:]:
                nc.sync.dma_start(xsb[:, c0g : c0g + gwg], xT[:, c0g : c0g + gwg])

            # ---- per-member pipeline stages ---------------------------------
            def emit_l1(uid, c0, gw, nt, e):
                # z1 = W1[e].T @ x  (K=32, M=256) -> PSUM
                h1ps = [
                    hps.tile([128, GWm], F32, tag="h", name=f"h1ps{uid}m{m}")
                    for m in range(2)
                ]
                for m in range(2):
                    lhsT = w1sb[:, e * H + m * 128 : e * H + (m + 1) * 128]
                    for t in range(nt):
                        ts = slice(t * BT, (t + 1) * BT)
                        nc.tensor.matmul(
                            h1ps[m][:, ts],
                            lhsT,
                            xsb[:, c0 + t * BT : c0 + (t + 1) * BT],
                            start=True,
                            stop=True,
                        )
                return h1ps

            def emit_act1(uid, gw, e, h1ps):
                # silu(z1 + b1) fused on ScalarE, PSUM -> SBUF(dt)
                h1sb = [
                    hpool.tile([128, GWm], dt, tag="hs", name=f"h1sb{uid}m{m}")
                    for m in range(2)
                ]
                for m in range(2):
                    nc.scalar.activation(
                        h1sb[m][:, :gw],
                        h1ps[m][:, :gw],
                        Silu,
                        bias=bsb[:, 2 * e + m : 2 * e + m + 1],
                    )
                return h1sb

            def emit_l2(uid, gw, nt, e, h1sb):
                # z2 = W2[e].T @ h1 (K=256 via 2 k-tiles) -> PSUM
                h2ps = [
                    hps.tile([128, GWm], F32, tag="h", name=f"h2ps{uid}m{m}")
                    for m in range(2)
                ]
                for m in range(2):
                    for k in range(2):
                        base = (2 * e + k) * H + m * 128
                        lhsT = w2sb[:, base : base + 128]
                        for t in range(nt):
                            ts = slice(t * BT, (t + 1) * BT)
                            nc.tensor.matmul(
                                h2ps[m][:, ts],
                                lhsT,
                                h1sb[k][:, ts],
                                start=(k == 0),
                                stop=(k == 1),
                            )
                return h2ps

            def emit_act2(uid, gw, e, h2ps):
                h2sb = [
                    hpool.tile([128, GWm], dt, tag="hs", name=f"h2sb{uid}m{m}")
                    for m in range(2)
                ]
                for m in range(2):
                    nc.scalar.activation(
                        h2sb[m][:, :gw],
                        h2ps[m][:, :gw],
                        Silu,
                        bias=bsb[:, 2 * E + 2 * e + m : 2 * E + 2 * e + m + 1],
                    )
                return h2sb

            def emit_l3(uid, c0, gw, e, h2sb):
                # pred = W3[e].T @ h2 (K=256, M=48), in chunks of pw columns
                for pc0 in range(0, gw, pw):
                    pcw = min(pw, gw - pc0)
                    pps = pred_pool.tile(
                        [OUT2, pw],
                        F32,
                        tag="h" if pred_in_h else "o",
                        name=f"pps{uid}c{pc0}",
                    )
                    for k in range(2):
                        base = (2 * e + k) * OUT2
                        lhsT = w3sb[:, base : base + OUT2]
                        for t in range(pcw // BT):
                            ts = slice(pc0 + t * BT, pc0 + (t + 1) * BT)
                            ps = slice(t * BT, (t + 1) * BT)
                            nc.tensor.matmul(
                                pps[:, ps],
                                lhsT,
                                h2sb[k][:, ts],
                                start=(k == 0),
                                stop=(k == 1),
                            )
                    # bias add on VectorE (PSUM -> SBUF fp32) + store
                    osb = opool.tile([OUT2, pw], F32, tag="os", name=f"osb{uid}c{pc0}")
                    nc.vector.tensor_scalar_add(
                        osb[:, :pcw],
                        pps[:, :pcw],
                        bsb[:OUT2, 4 * E + e : 4 * E + e + 1],
                    )
                    nc.sync.dma_start(
                        out[e, :, c0 + pc0 : c0 + pc0 + pcw], osb[:, :pcw]
                    )

            def body(rep):
                for g, (c0, gw) in enumerate(groups):
                    nt = gw // BT
                    # interleave `ilv` ensemble members so every ACT input was
                    # produced >=2 ACT-instructions earlier (no pipeline bubble)
                    for e0 in range(0, E, ilv):
                        es = list(range(e0, min(e0 + ilv, E)))
                        uids = {e: f"r{rep}g{g}e{e}" for e in es}
                        h1ps = {e: emit_l1(uids[e], c0, gw, nt, e) for e in es}
                        h1sb = {e: emit_act1(uids[e], gw, e, h1ps[e]) for e in es}
                        h2ps = {e: emit_l2(uids[e], gw, nt, e, h1sb[e]) for e in es}
                        h2sb = {e: emit_act2(uids[e], gw, e, h2ps[e]) for e in es}
                        for e in es:
                            emit_l3(uids[e], c0, gw, e, h2sb[e])

            if inner_reps == 1:
                body(0)
            else:
                # hardware loop for timing: rerun the whole computation
                # inner_reps times per launch (outputs just get overwritten)
                ET = mybir.EngineType
                with tc.For_i(
                    0,
                    inner_reps,
                    1,
                    hint_engines=(ET.PE, ET.Activation, ET.DVE, ET.SP, ET.Pool),
                ):
                    body(0)

    nc.finalize()
    return nc


def prep_inputs(x, W1, b1, W2, b2, W3, b3, compute="bf16"):
    """Host-side shard + repack into the per-core DRAM layouts."""
    npdt = ml_dtypes.bfloat16 if compute == "bf16" else np.float32
    x = np.asarray(x, np.float32)
    W1 = np.asarray(W1, np.float32)
    W2 = np.asarray(W2, np.float32)
    W3 = np.asarray(W3, np.float32)
    b1 = np.asarray(b1, np.float32)
    b2 = np.asarray(b2, np.float32)
    b3 = np.asarray(b3, np.float32)

    # weights: [K, M] stationary layouts, shared by all cores
    w1h = np.ascontiguousarray(W1.transpose(1, 0, 2).reshape(IN, E * H)).astype(npdt)
    w2h = np.ascontiguousarray(
        W2.reshape(E, 2, 128, H).transpose(2, 0, 1, 3).reshape(128, E * 2 * H)
    ).astype(npdt)
    w3h = np.ascontiguousarray(
        W3.reshape(E, 2, 128, OUT2).transpose(2, 0, 1, 3).reshape(128, E * 2 * OUT2)
    ).astype(npdt)

    # biases: [128, 5E] fp32; col 2e+m -> b1[e] m-half, col 2E+2e+m -> b2,
    # col 4E+e -> b3[e] in rows 0:48
    bsh = np.zeros((128, 5 * E), np.float32)
    for e in range(E):
        for m in range(2):
            bsh[:, 2 * e + m] = b1[e, m * 128 : (m + 1) * 128]
            bsh[:, 2 * E + 2 * e + m] = b2[e, m * 128 : (m + 1) * 128]
        bsh[:OUT2, 4 * E + e] = b3[e]

    in_maps = []
    xs = x.reshape(NCORES, BL, IN)
    for c in range(NCORES):
        xTc = np.ascontiguousarray(xs[c].T).astype(npdt)  # [IN, BL]
        in_maps.append(
            {"xT": xTc, "w1": w1h, "w2": w2h, "w3": w3h, "bs": bsh}
        )
    return in_maps


def assemble(results):
    """Per-core [E, 48, BL] fp32 -> (means, logvars) [E, B, 24] fp32."""
    full = np.concatenate([r["out"] for r in results], axis=2)  # [E, 48, B]
    pred = np.ascontiguousarray(full.transpose(0, 2, 1))        # [E, B, 48]
    means = np.ascontiguousarray(pred[:, :, :OUT])
    logvars = np.ascontiguousarray(pred[:, :, OUT:])
    return means, logvars


_NC_CACHE = {}


def _get_nc(compute, inner_reps=1, **kw):
    key = (compute, inner_reps, tuple(sorted(kw.items())))
    if key not in _NC_CACHE:
        _NC_CACHE[key] = build_nc(compute, inner_reps, **kw)
    return _NC_CACHE[key]


def kernel(x, W1, b1, W2, b2, W3, b3):
    compute = COMPUTE_DTYPE
    nc = _get_nc(compute, 1, **BEST_KW)
    in_maps = prep_inputs(x, W1, b1, W2, b2, W3, b3, compute)
    res = run_bass_kernel_spmd(nc, in_maps, core_ids=list(range(NCORES)))
    return assemble(res.results)


# revision 33
# speedup vs baseline: 1.2550x; 1.2550x over previous
"""Trainium2 Bass kernel for nn_EnsembleModel (ensemble MLP, E=10).

Computes, for each ensemble member e:
    h1 = silu(x @ W1[e] + b1[e])      # [B, 256]
    h2 = silu(h1 @ W2[e] + b2[e])     # [B, 256]
    pred = h2 @ W3[e] + b3[e]         # [B, 48]
    means, logvars = pred[:, :24], pred[:, 24:]

Sharding: data-parallel over the batch axis across 8 NeuronCores; all
ensemble weights are replicated and SBUF-resident on every core.

On-chip layout: features live on SBUF partitions, batch on the free dim.
Matmuls are W.T @ x with the weight stationary ([K, M]) and the batch
streaming as the moving operand (N=512 per PSUM bank).  Silu+bias is fused
into one ScalarE activation per [128, 1024] tile reading PSUM directly.
"""

import numpy as np
import ml_dtypes

import concourse.bacc as bacc
import concourse.mybir as mybir
import concourse.tile as tile
from concourse.bass_utils import run_bass_kernel_spmd

# problem dims (hardcoded per harness contract)
E = 10          # ensemble members
IN = 32         # input dim
H = 256         # hidden
OUT = 24        # output dim (head emits 2*OUT)
OUT2 = 2 * OUT  # 48
B = 65536
NCORES = 8
BL = B // NCORES    # 8192 batch rows per core
BT = 512            # matmul moving free dim (one fp32 PSUM bank)
GRP = 2             # batch tiles per iteration group
GW = BT * GRP       # 1024
NGRP = BL // GW     # 8

F32 = mybir.dt.float32

# compute dtype for matmul operands:
#   "f32r": fp32 storage, relaxed-precision matmul (TF32-ish) — same PE rate
#           as bf16 for N>=256, rel err ~2.5e-4 vs the fp32 reference
#   "bf16": fastest/smallest, rel err ~4e-3
#   "f32":  exact but 4x slower matmuls
COMPUTE_DTYPE = "f32r"

# best-known kernel structure (chosen via TimelineSim + HW timing sweeps)
BEST_KW = dict(ilv=2, l1pack=True, hsbufs=8, pw=512, obufs=2)


_DTYPES = {
    "bf16": mybir.dt.bfloat16,
    "f32": mybir.dt.float32,
    "f32r": mybir.dt.float32r,
}


def build_nc(
    compute="bf16",
    inner_reps=1,
    grp=GRP,
    hbufs=3,
    pred_in_h=False,
    pw=None,
    obufs=1,
    ilv=1,
    l1pack=False,
    hsbufs=6,
    warm_groups=0,
    act_split=0,
):
    """Build the per-core Bass program (SPMD: same NEFF on all 8 cores)."""
    dt = _DTYPES[compute]
    nc = bacc.Bacc("TRN2", target_bir_lowering=False, debug=False)

    # batch groups per member: list of (col offset, width); width = grp*BT
    # except a possibly smaller tail group.  The first `warm_groups` groups
    # are single-BT so the pipeline fills faster at kernel start.
    groups = []
    off = 0
    for _ in range(warm_groups):
        groups.append((off, BT))
        off += BT
    while off < BL:
        w = min(grp * BT, BL - off)
        groups.append((off, w))
        off += w

    xT_d = nc.dram_tensor("xT", [IN, BL], dt, kind="ExternalInput")
    w1_shape = [2 * IN, E * 128] if l1pack else [IN, E * H]
    w1_d = nc.dram_tensor("w1", w1_shape, dt, kind="ExternalInput")
    w2_d = nc.dram_tensor("w2", [128, E * 2 * H], dt, kind="ExternalInput")
    w3_d = nc.dram_tensor("w3", [128, E * 2 * OUT2], dt, kind="ExternalInput")
    bs_d = nc.dram_tensor("bs", [128, 5 * E], F32, kind="ExternalInput")
    out_d = nc.dram_tensor("out", [E, OUT2, BL], F32, kind="ExternalOutput")

    xT = xT_d.ap()
    w1 = w1_d.ap()
    w2 = w2_d.ap()
    w3 = w3_d.ap()
    bs = bs_d.ap()
    out = out_d.ap()

    Silu = mybir.ActivationFunctionType.Silu

    GWm = grp * BT  # max group width
    if pw is None:
        pw = GWm  # pred-psum chunk width
    assert pw % BT == 0

    with tile.TileContext(nc) as tc:
        with (
            tc.tile_pool(name="consts", bufs=1) as cpool,
            tc.tile_pool(name="hsb", bufs=hsbufs) as hpool,
            tc.tile_pool(name="osb", bufs=2) as opool,
            tc.tile_pool(name="hps", bufs=hbufs, space="PSUM") as hps,
            tc.tile_pool(name="ops", bufs=obufs, space="PSUM") as ops,
        ):
            pred_pool = hps if pred_in_h else ops
            xsb = cpool.tile([2 * IN if l1pack else IN, BL], dt, name="xsb")
            w1sb = cpool.tile(w1_shape, dt, name="w1sb")
            w2sb = cpool.tile([128, E * 2 * H], dt, name="w2sb")
            w3sb = cpool.tile([128, E * 2 * OUT2], dt, name="w3sb")
            bsb = cpool.tile([128, 5 * E], F32, name="bsb")
            # loads ordered / chunked so the first (g0, e0) iteration's
            # dependencies arrive first and compute can start early
            def load_x(c0g, gwg):
                nc.sync.dma_start(xsb[:IN, c0g : c0g + gwg], xT[:, c0g : c0g + gwg])
                if l1pack:
                    # replicate x into partitions 32:64 so the two L1 m-tiles
                    # can run concurrently in different PE row groups
                    nc.sync.dma_start(
                        xsb[IN : 2 * IN, c0g : c0g + gwg], xT[:, c0g : c0g + gwg]
                    )

            nc.sync.dma_start(bsb[:], bs)
            load_x(*groups[0])
            w1w = 128 if l1pack else H
            for e in range(E):
                nc.sync.dma_start(
                    w1sb[:, e * w1w : (e + 1) * w1w], w1[:, e * w1w : (e + 1) * w1w]
                )
                nc.sync.dma_start(
                    w2sb[:, e * 2 * H : (e + 1) * 2 * H],
                    w2[:, e * 2 * H : (e + 1) * 2 * H],
                )
                nc.sync.dma_start(
                    w3sb[:, e * 2 * OUT2 : (e + 1) * 2 * OUT2],
                    w3[:, e * 2 * OUT2 : (e + 1) * 2 * OUT2],
                )
            for c0g, gwg in groups[1:]:
                load_x(c0g, gwg)

            # ---- per-member pipeline stages ---------------------------------
            def emit_l1(uid, c0, gw, nt, e):
                # z1 = W1[e].T @ x  (K=32, M=256) -> PSUM
                h1ps = [
                    hps.tile([128, GWm], F32, tag="h", name=f"h1ps{uid}m{m}")
                    for m in range(2)
                ]
                for m in range(2):
                    if l1pack:
                        # m-tile m in PE row group m (tile_position auto-derived
                        # from base partition) -> the two MMs run concurrently
                        lhsT = w1sb[m * IN : (m + 1) * IN, e * 128 : (e + 1) * 128]
                        rhs_rows = slice(m * IN, (m + 1) * IN)
                    else:
                        lhsT = w1sb[:, e * H + m * 128 : e * H + (m + 1) * 128]
                        rhs_rows = slice(0, IN)
                    for t in range(nt):
                        ts = slice(t * BT, (t + 1) * BT)
                        nc.tensor.matmul(
                            h1ps[m][:, ts],
                            lhsT,
                            xsb[rhs_rows, c0 + t * BT : c0 + (t + 1) * BT],
                            start=True,
                            stop=True,
                        )
                return h1ps

            def emit_act(uid, gw, h_ps, bias_col0, layer):
                # silu(z + b) fused on ScalarE, PSUM -> SBUF(dt); optionally
                # split into act_split-wide chunks (e.g. one PSUM bank each)
                h_sb = [
                    hpool.tile([128, GWm], dt, tag="hs", name=f"h{layer}sb{uid}m{m}")
                    for m in range(2)
                ]
                aw = act_split if act_split else gw
                for m in range(2):
                    for a0 in range(0, gw, aw):
                        a1 = min(a0 + aw, gw)
                        nc.scalar.activation(
                            h_sb[m][:, a0:a1],
                            h_ps[m][:, a0:a1],
                            Silu,
                            bias=bsb[:, bias_col0 + m : bias_col0 + m + 1],
                        )
                return h_sb

            def emit_act1(uid, gw, e, h1ps):
                return emit_act(uid, gw, h1ps, 2 * e, 1)

            def emit_l2(uid, gw, nt, e, h1sb):
                # z2 = W2[e].T @ h1 (K=256 via 2 k-tiles) -> PSUM
                h2ps = [
                    hps.tile([128, GWm], F32, tag="h", name=f"h2ps{uid}m{m}")
                    for m in range(2)
                ]
                for m in range(2):
                    for k in range(2):
                        base = (2 * e + k) * H + m * 128
                        lhsT = w2sb[:, base : base + 128]
                        for t in range(nt):
                            ts = slice(t * BT, (t + 1) * BT)
                            nc.tensor.matmul(
                                h2ps[m][:, ts],
                                lhsT,
                                h1sb[k][:, ts],
                                start=(k == 0),
                                stop=(k == 1),
                            )
                return h2ps

            def emit_act2(uid, gw, e, h2ps):
                return emit_act(uid, gw, h2ps, 2 * E + 2 * e, 2)

            def emit_l3(uid, c0, gw, e, h2sb):
                # pred = W3[e].T @ h2 (K=256, M=48), in chunks of pw columns
                for pc0 in range(0, gw, pw):
                    pcw = min(pw, gw - pc0)
                    pps = pred_pool.tile(
                        [OUT2, pw],
                        F32,
                        tag="h" if pred_in_h else "o",
                        name=f"pps{uid}c{pc0}",
                    )
                    for k in range(2):
                        base = (2 * e + k) * OUT2
                        lhsT = w3sb[:, base : base + OUT2]
                        for t in range(pcw // BT):
                            ts = slice(pc0 + t * BT, pc0 + (t + 1) * BT)
                            ps = slice(t * BT, (t + 1) * BT)
                            nc.tensor.matmul(
                                pps[:, ps],
                                lhsT,
                                h2sb[k][:, ts],
                                start=(k == 0),
                                stop=(k == 1),
                            )
                    # bias add on VectorE (PSUM -> SBUF fp32) + store
                    osb = opool.tile([OUT2, pw], F32, tag="os", name=f"osb{uid}c{pc0}")
                    nc.vector.tensor_scalar_add(
                        osb[:, :pcw],
                        pps[:, :pcw],
                        bsb[:OUT2, 4 * E + e : 4 * E + e + 1],
                    )
                    nc.sync.dma_start(
                        out[e, :, c0 + pc0 : c0 + pc0 + pcw], osb[:, :pcw]
                    )

            def body(rep):
                for g, (c0, gw) in enumerate(groups):
                    nt = gw // BT
                    # interleave `ilv` ensemble members so every ACT input was
                    # produced >=2 ACT-instructions earlier (no pipeline bubble)
                    for e0 in range(0, E, ilv):
                        es = list(range(e0, min(e0 + ilv, E)))
                        uids = {e: f"r{rep}g{g}e{e}" for e in es}
                        h1ps = {e: emit_l1(uids[e], c0, gw, nt, e) for e in es}
                        h1sb = {e: emit_act1(uids[e], gw, e, h1ps[e]) for e in es}
                        h2ps = {e: emit_l2(uids[e], gw, nt, e, h1sb[e]) for e in es}
                        h2sb = {e: emit_act2(uids[e], gw, e, h2ps[e]) for e in es}
                        for e in es:
                            emit_l3(uids[e], c0, gw, e, h2sb[e])

            if inner_reps == 1:
                body(0)
            else:
                # hardware loop for timing: rerun the whole computation
                # inner_reps times per launch (outputs just get overwritten)
                ET = mybir.EngineType
                with tc.For_i(
                    0,
                    inner_reps,
                    1,
                    hint_engines=(ET.PE, ET.Activation, ET.DVE, ET.SP, ET.Pool),
                ):
                    body(0)

    nc.finalize()
    return nc


def prep_inputs(x, W1, b1, W2, b2, W3, b3, compute="bf16", l1pack=False):
    """Host-side shard + repack into the per-core DRAM layouts."""
    npdt = ml_dtypes.bfloat16 if compute == "bf16" else np.float32  # f32r is f32 bits
    x = np.asarray(x, np.float32)
    W1 = np.asarray(W1, np.float32)
    W2 = np.asarray(W2, np.float32)
    W3 = np.asarray(W3, np.float32)
    b1 = np.asarray(b1, np.float32)
    b2 = np.asarray(b2, np.float32)
    b3 = np.asarray(b3, np.float32)

    # weights: [K, M] stationary layouts, shared by all cores
    if l1pack:
        # [64, E*128]: rows 0:32 = m-tile 0, rows 32:64 = m-tile 1
        w1h = np.zeros((2 * IN, E * 128), np.float32)
        for e in range(E):
            w1h[:IN, e * 128 : (e + 1) * 128] = W1[e][:, :128]
            w1h[IN:, e * 128 : (e + 1) * 128] = W1[e][:, 128:]
        w1h = w1h.astype(npdt)
    else:
        w1h = np.ascontiguousarray(W1.transpose(1, 0, 2).reshape(IN, E * H)).astype(
            npdt
        )
    w2h = np.ascontiguousarray(
        W2.reshape(E, 2, 128, H).transpose(2, 0, 1, 3).reshape(128, E * 2 * H)
    ).astype(npdt)
    w3h = np.ascontiguousarray(
        W3.reshape(E, 2, 128, OUT2).transpose(2, 0, 1, 3).reshape(128, E * 2 * OUT2)
    ).astype(npdt)

    # biases: [128, 5E] fp32; col 2e+m -> b1[e] m-half, col 2E+2e+m -> b2,
    # col 4E+e -> b3[e] in rows 0:48
    bsh = np.zeros((128, 5 * E), np.float32)
    for e in range(E):
        for m in range(2):
            bsh[:, 2 * e + m] = b1[e, m * 128 : (m + 1) * 128]
            bsh[:, 2 * E + 2 * e + m] = b2[e, m * 128 : (m + 1) * 128]
        bsh[:OUT2, 4 * E + e] = b3[e]

    in_maps = []
    xs = x.reshape(NCORES, BL, IN)
    for c in range(NCORES):
        xTc = np.ascontiguousarray(xs[c].T).astype(npdt)  # [IN, BL]
        in_maps.append(
            {"xT": xTc, "w1": w1h, "w2": w2h, "w3": w3h, "bs": bsh}
        )
    return in_maps


def assemble(results):
    """Per-core [E, 48, BL] fp32 -> (means, logvars) [E, B, 24] fp32."""
    full = np.concatenate([r["out"] for r in results], axis=2)  # [E, 48, B]
    pred = np.ascontiguousarray(full.transpose(0, 2, 1))        # [E, B, 48]
    means = np.ascontiguousarray(pred[:, :, :OUT])
    logvars = np.ascontiguousarray(pred[:, :, OUT:])
    return means, logvars


_NC_CACHE = {}


def _get_nc(compute, inner_reps=1, **kw):
    key = (compute, inner_reps, tuple(sorted(kw.items())))
    if key not in _NC_CACHE:
        _NC_CACHE[key] = build_nc(compute, inner_reps, **kw)
    return _NC_CACHE[key]


def kernel(x, W1, b1, W2, b2, W3, b3):
    compute = COMPUTE_DTYPE
    nc = _get_nc(compute, 1, **BEST_KW)
    in_maps = prep_inputs(
        x, W1, b1, W2, b2, W3, b3, compute, l1pack=BEST_KW.get("l1pack", False)
    )
    res = run_bass_kernel_spmd(nc, in_maps, core_ids=list(range(NCORES)))
    return assemble(res.results)


# revision 43
# speedup vs baseline: 1.2793x; 1.0194x over previous
"""Trainium2 Bass kernel for nn_EnsembleModel (ensemble MLP, E=10).

Computes, for each ensemble member e:
    h1 = silu(x @ W1[e] + b1[e])      # [B, 256]
    h2 = silu(h1 @ W2[e] + b2[e])     # [B, 256]
    pred = h2 @ W3[e] + b3[e]         # [B, 48]
    means, logvars = pred[:, :24], pred[:, 24:]

Sharding: data-parallel over the batch axis across 8 NeuronCores; all
ensemble weights are replicated and SBUF-resident on every core.

On-chip layout: features live on SBUF partitions, batch on the free dim.
Matmuls are W.T @ x with the weight stationary ([K, M]) and the batch
streaming as the moving operand (N=512 per PSUM bank).  Silu+bias is fused
into one ScalarE activation per [128, 1024] tile reading PSUM directly.
"""

import numpy as np
import ml_dtypes

import concourse.bacc as bacc
import concourse.mybir as mybir
import concourse.tile as tile
from concourse.bass_utils import run_bass_kernel_spmd

# problem dims (hardcoded per harness contract)
E = 10          # ensemble members
IN = 32         # input dim
H = 256         # hidden
OUT = 24        # output dim (head emits 2*OUT)
OUT2 = 2 * OUT  # 48
B = 65536
NCORES = 8
BL = B // NCORES    # 8192 batch rows per core
BT = 512            # matmul moving free dim (one fp32 PSUM bank)
GRP = 2             # batch tiles per iteration group
GW = BT * GRP       # 1024
NGRP = BL // GW     # 8

F32 = mybir.dt.float32

# compute dtype for matmul operands:
#   "f32r": fp32 storage, relaxed-precision matmul (TF32-ish) — same PE rate
#           as bf16 for N>=256, rel err ~2.5e-4 vs the fp32 reference
#   "bf16": fastest/smallest, rel err ~4e-3
#   "f32":  exact but 4x slower matmuls
COMPUTE_DTYPE = "f32r"

# best-known kernel structure (chosen via TimelineSim + HW timing sweeps)
BEST_KW = dict(ilv=2, l1pack=True, hsbufs=8, pw=512, obufs=2)


_DTYPES = {
    "bf16": mybir.dt.bfloat16,
    "f32": mybir.dt.float32,
    "f32r": mybir.dt.float32r,
}


def build_nc(
    compute="bf16",
    inner_reps=1,
    grp=GRP,
    hbufs=3,
    pred_in_h=False,
    pw=None,
    obufs=1,
    ilv=1,
    l1pack=False,
    hsbufs=6,
    warm_groups=0,
    act_split=0,
    l3pack=False,
):
    """Build the per-core Bass program (SPMD: same NEFF on all 8 cores)."""
    dt = _DTYPES[compute]
    nc = bacc.Bacc("TRN2", target_bir_lowering=False, debug=False)

    # batch groups per member: list of (col offset, width); width = grp*BT
    # except a possibly smaller tail group.  The first `warm_groups` groups
    # are single-BT so the pipeline fills faster at kernel start.
    groups = []
    off = 0
    for _ in range(warm_groups):
        groups.append((off, BT))
        off += BT
    while off < BL:
        w = min(grp * BT, BL - off)
        groups.append((off, w))
        off += w

    xT_d = nc.dram_tensor("xT", [IN, BL], dt, kind="ExternalInput")
    w1_shape = [2 * IN, E * 128] if l1pack else [IN, E * H]
    w1_d = nc.dram_tensor("w1", w1_shape, dt, kind="ExternalInput")
    w2_d = nc.dram_tensor("w2", [128, E * 2 * H], dt, kind="ExternalInput")
    # l3pack pads each W3 k-tile from 48 to 64 columns (zeros) so the packed
    # matmul outputs land on 64-aligned PE column groups
    OW = 64 if l3pack else OUT2
    BW = 5 * E + (E // 2 if l3pack else 0)
    w3_d = nc.dram_tensor("w3", [128, E * 2 * OW], dt, kind="ExternalInput")
    bs_d = nc.dram_tensor("bs", [128, BW], F32, kind="ExternalInput")
    out_d = nc.dram_tensor("out", [E, OUT2, BL], F32, kind="ExternalOutput")

    xT = xT_d.ap()
    w1 = w1_d.ap()
    w2 = w2_d.ap()
    w3 = w3_d.ap()
    bs = bs_d.ap()
    out = out_d.ap()

    Silu = mybir.ActivationFunctionType.Silu

    GWm = grp * BT  # max group width
    if pw is None:
        pw = GWm  # pred-psum chunk width
    assert pw % BT == 0

    with tile.TileContext(nc) as tc:
        with (
            tc.tile_pool(name="consts", bufs=1) as cpool,
            tc.tile_pool(name="hsb", bufs=hsbufs) as hpool,
            tc.tile_pool(name="osb", bufs=2) as opool,
            tc.tile_pool(name="hps", bufs=hbufs, space="PSUM") as hps,
            tc.tile_pool(name="ops", bufs=obufs, space="PSUM") as ops,
        ):
            pred_pool = hps if pred_in_h else ops
            xsb = cpool.tile([2 * IN if l1pack else IN, BL], dt, name="xsb")
            w1sb = cpool.tile(w1_shape, dt, name="w1sb")
            w2sb = cpool.tile([128, E * 2 * H], dt, name="w2sb")
            w3sb = cpool.tile([128, E * 2 * OW], dt, name="w3sb")
            bsb = cpool.tile([128, BW], F32, name="bsb")
            # loads ordered / chunked so the first (g0, e0) iteration's
            # dependencies arrive first and compute can start early
            def load_x(c0g, gwg):
                nc.sync.dma_start(xsb[:IN, c0g : c0g + gwg], xT[:, c0g : c0g + gwg])
                if l1pack:
                    # replicate x into partitions 32:64 so the two L1 m-tiles
                    # can run concurrently in different PE row groups
                    nc.sync.dma_start(
                        xsb[IN : 2 * IN, c0g : c0g + gwg], xT[:, c0g : c0g + gwg]
                    )

            nc.sync.dma_start(bsb[:], bs)
            load_x(*groups[0])
            w1w = 128 if l1pack else H
            for e in range(E):
                nc.sync.dma_start(
                    w1sb[:, e * w1w : (e + 1) * w1w], w1[:, e * w1w : (e + 1) * w1w]
                )
                nc.sync.dma_start(
                    w2sb[:, e * 2 * H : (e + 1) * 2 * H],
                    w2[:, e * 2 * H : (e + 1) * 2 * H],
                )
                nc.sync.dma_start(
                    w3sb[:, e * 2 * OW : (e + 1) * 2 * OW],
                    w3[:, e * 2 * OW : (e + 1) * 2 * OW],
                )
            for c0g, gwg in groups[1:]:
                load_x(c0g, gwg)

            # ---- per-member pipeline stages ---------------------------------
            def emit_l1(uid, c0, gw, nt, e):
                # z1 = W1[e].T @ x  (K=32, M=256) -> PSUM
                h1ps = [
                    hps.tile([128, GWm], F32, tag="h", name=f"h1ps{uid}m{m}")
                    for m in range(2)
                ]
                for m in range(2):
                    if l1pack:
                        # m-tile m in PE row group m (tile_position auto-derived
                        # from base partition) -> the two MMs run concurrently
                        lhsT = w1sb[m * IN : (m + 1) * IN, e * 128 : (e + 1) * 128]
                        rhs_rows = slice(m * IN, (m + 1) * IN)
                    else:
                        lhsT = w1sb[:, e * H + m * 128 : e * H + (m + 1) * 128]
                        rhs_rows = slice(0, IN)
                    for t in range(nt):
                        ts = slice(t * BT, (t + 1) * BT)
                        nc.tensor.matmul(
                            h1ps[m][:, ts],
                            lhsT,
                            xsb[rhs_rows, c0 + t * BT : c0 + (t + 1) * BT],
                            start=True,
                            stop=True,
                        )
                return h1ps

            def emit_act(uid, gw, h_ps, bias_col0, layer):
                # silu(z + b) fused on ScalarE, PSUM -> SBUF(dt); optionally
                # split into act_split-wide chunks (e.g. one PSUM bank each)
                h_sb = [
                    hpool.tile([128, GWm], dt, tag="hs", name=f"h{layer}sb{uid}m{m}")
                    for m in range(2)
                ]
                aw = act_split if act_split else gw
                for m in range(2):
                    for a0 in range(0, gw, aw):
                        a1 = min(a0 + aw, gw)
                        nc.scalar.activation(
                            h_sb[m][:, a0:a1],
                            h_ps[m][:, a0:a1],
                            Silu,
                            bias=bsb[:, bias_col0 + m : bias_col0 + m + 1],
                        )
                return h_sb

            def emit_act1(uid, gw, e, h1ps):
                return emit_act(uid, gw, h1ps, 2 * e, 1)

            def emit_l2(uid, gw, nt, e, h1sb):
                # z2 = W2[e].T @ h1 (K=256 via 2 k-tiles) -> PSUM
                h2ps = [
                    hps.tile([128, GWm], F32, tag="h", name=f"h2ps{uid}m{m}")
                    for m in range(2)
                ]
                for m in range(2):
                    for k in range(2):
                        base = (2 * e + k) * H + m * 128
                        lhsT = w2sb[:, base : base + 128]
                        for t in range(nt):
                            ts = slice(t * BT, (t + 1) * BT)
                            nc.tensor.matmul(
                                h2ps[m][:, ts],
                                lhsT,
                                h1sb[k][:, ts],
                                start=(k == 0),
                                stop=(k == 1),
                            )
                return h2ps

            def emit_act2(uid, gw, e, h2ps):
                return emit_act(uid, gw, h2ps, 2 * E + 2 * e, 2)

            def emit_l3(uid, c0, gw, e, h2sb):
                # pred = W3[e].T @ h2 (K=256, M=48), in chunks of pw columns
                for pc0 in range(0, gw, pw):
                    pcw = min(pw, gw - pc0)
                    pps = pred_pool.tile(
                        [OUT2, pw],
                        F32,
                        tag="h" if pred_in_h else "o",
                        name=f"pps{uid}c{pc0}",
                    )
                    for k in range(2):
                        base = (2 * e + k) * OW
                        lhsT = w3sb[:, base : base + OUT2]
                        for t in range(pcw // BT):
                            ts = slice(pc0 + t * BT, pc0 + (t + 1) * BT)
                            ps = slice(t * BT, (t + 1) * BT)
                            nc.tensor.matmul(
                                pps[:, ps],
                                lhsT,
                                h2sb[k][:, ts],
                                start=(k == 0),
                                stop=(k == 1),
                            )
                    # bias add on VectorE (PSUM -> SBUF fp32) + store
                    osb = opool.tile([OUT2, pw], F32, tag="os", name=f"osb{uid}c{pc0}")
                    nc.vector.tensor_scalar_add(
                        osb[:, :pcw],
                        pps[:, :pcw],
                        bsb[:OUT2, 4 * E + e : 4 * E + e + 1],
                    )
                    nc.sync.dma_start(
                        out[e, :, c0 + pc0 : c0 + pc0 + pcw], osb[:, :pcw]
                    )

            def emit_l3_pair(uid, c0, gw, ea, eb, h2a, h2b):
                # both members' L3 packed into one pred tile: ea in PE column
                # groups 0-1 (partitions 0:48), eb in groups 2-3 (64:112).
                # The MMs run concurrently in the array.  Only the very first
                # matmul per bank may use start=True (it clears the whole
                # bank's has_written bits).
                for pc0 in range(0, gw, pw):
                    pcw = min(pw, gw - pc0)
                    pps = pred_pool.tile(
                        [128, pw], F32, tag="o", name=f"pps{uid}c{pc0}"
                    )
                    first = True
                    for k in range(2):
                        for e, h2, rb in ((ea, h2a, 0), (eb, h2b, 64)):
                            lhsT = w3sb[:, (2 * e + k) * OW : (2 * e + k) * OW + OW]
                            for t in range(pcw // BT):
                                ts = slice(pc0 + t * BT, pc0 + (t + 1) * BT)
                                ps = slice(t * BT, (t + 1) * BT)
                                nc.tensor.matmul(
                                    pps[rb : rb + OW, ps],
                                    lhsT,
                                    h2[k][:, ts],
                                    start=first,
                                    stop=(k == 1),
                                    skip_group_check=True,
                                )
                                first = False
                    osb = opool.tile([112, pw], F32, tag="os", name=f"osb{uid}c{pc0}")
                    for e, rb in ((ea, 0), (eb, 64)):
                        nc.vector.tensor_scalar_add(
                            osb[rb : rb + OUT2, :pcw],
                            pps[rb : rb + OUT2, :pcw],
                            bsb[rb : rb + OUT2, 5 * E + ea // 2 : 5 * E + ea // 2 + 1],
                        )
                        nc.sync.dma_start(
                            out[e, :, c0 + pc0 : c0 + pc0 + pcw],
                            osb[rb : rb + OUT2, :pcw],
                        )

            def body(rep):
                for g, (c0, gw) in enumerate(groups):
                    nt = gw // BT
                    # interleave `ilv` ensemble members so every ACT input was
                    # produced >=2 ACT-instructions earlier (no pipeline bubble)
                    for e0 in range(0, E, ilv):
                        es = list(range(e0, min(e0 + ilv, E)))
                        uids = {e: f"r{rep}g{g}e{e}" for e in es}
                        h1ps = {e: emit_l1(uids[e], c0, gw, nt, e) for e in es}
                        h1sb = {e: emit_act1(uids[e], gw, e, h1ps[e]) for e in es}
                        h2ps = {e: emit_l2(uids[e], gw, nt, e, h1sb[e]) for e in es}
                        h2sb = {e: emit_act2(uids[e], gw, e, h2ps[e]) for e in es}
                        if l3pack and len(es) == 2:
                            ea, eb = es
                            emit_l3_pair(
                                uids[ea], c0, gw, ea, eb, h2sb[ea], h2sb[eb]
                            )
                        else:
                            for e in es:
                                emit_l3(uids[e], c0, gw, e, h2sb[e])

            if inner_reps == 1:
                body(0)
            else:
                # hardware loop for timing: rerun the whole computation
                # inner_reps times per launch (outputs just get overwritten)
                ET = mybir.EngineType
                with tc.For_i(
                    0,
                    inner_reps,
                    1,
                    hint_engines=(ET.PE, ET.Activation, ET.DVE, ET.SP, ET.Pool),
                ):
                    body(0)

    nc.finalize()
    return nc


def prep_inputs(x, W1, b1, W2, b2, W3, b3, compute="bf16", l1pack=False, l3pack=False):
    """Host-side shard + repack into the per-core DRAM layouts."""
    npdt = ml_dtypes.bfloat16 if compute == "bf16" else np.float32  # f32r is f32 bits
    x = np.asarray(x, np.float32)
    W1 = np.asarray(W1, np.float32)
    W2 = np.asarray(W2, np.float32)
    W3 = np.asarray(W3, np.float32)
    b1 = np.asarray(b1, np.float32)
    b2 = np.asarray(b2, np.float32)
    b3 = np.asarray(b3, np.float32)

    # weights: [K, M] stationary layouts, shared by all cores
    if l1pack:
        # [64, E*128]: rows 0:32 = m-tile 0, rows 32:64 = m-tile 1
        w1h = np.zeros((2 * IN, E * 128), np.float32)
        for e in range(E):
            w1h[:IN, e * 128 : (e + 1) * 128] = W1[e][:, :128]
            w1h[IN:, e * 128 : (e + 1) * 128] = W1[e][:, 128:]
        w1h = w1h.astype(npdt)
    else:
        w1h = np.ascontiguousarray(W1.transpose(1, 0, 2).reshape(IN, E * H)).astype(
            npdt
        )
    w2h = np.ascontiguousarray(
        W2.reshape(E, 2, 128, H).transpose(2, 0, 1, 3).reshape(128, E * 2 * H)
    ).astype(npdt)
    if l3pack:
        # pad each [128, 48] k-tile to [128, 64] with zero columns
        w3p = np.zeros((E, 2, 128, 64), np.float32)
        w3p[:, :, :, :OUT2] = W3.reshape(E, 2, 128, OUT2)
        w3h = np.ascontiguousarray(
            w3p.transpose(2, 0, 1, 3).reshape(128, E * 2 * 64)
        ).astype(npdt)
    else:
        w3h = np.ascontiguousarray(
            W3.reshape(E, 2, 128, OUT2).transpose(2, 0, 1, 3).reshape(128, E * 2 * OUT2)
        ).astype(npdt)

    # biases: [128, 5E + E/2] fp32; col 2e+m -> b1[e] m-half, col 2E+2e+m ->
    # b2, col 4E+e -> b3[e] in rows 0:48, col 5E+p -> the (2p, 2p+1) pair's
    # b3 in rows 0:48 / 64:112 (for the column-packed L3 path)
    bsh = np.zeros((128, 5 * E + (E // 2 if l3pack else 0)), np.float32)
    for e in range(E):
        for m in range(2):
            bsh[:, 2 * e + m] = b1[e, m * 128 : (m + 1) * 128]
            bsh[:, 2 * E + 2 * e + m] = b2[e, m * 128 : (m + 1) * 128]
        bsh[:OUT2, 4 * E + e] = b3[e]
    if l3pack:
        for p in range(E // 2):
            bsh[:OUT2, 5 * E + p] = b3[2 * p]
            bsh[64 : 64 + OUT2, 5 * E + p] = b3[2 * p + 1]

    in_maps = []
    xs = x.reshape(NCORES, BL, IN)
    for c in range(NCORES):
        xTc = np.ascontiguousarray(xs[c].T).astype(npdt)  # [IN, BL]
        in_maps.append(
            {"xT": xTc, "w1": w1h, "w2": w2h, "w3": w3h, "bs": bsh}
        )
    return in_maps


def assemble(results):
    """Per-core [E, 48, BL] fp32 -> (means, logvars) [E, B, 24] fp32."""
    full = np.concatenate([r["out"] for r in results], axis=2)  # [E, 48, B]
    pred = np.ascontiguousarray(full.transpose(0, 2, 1))        # [E, B, 48]
    means = np.ascontiguousarray(pred[:, :, :OUT])
    logvars = np.ascontiguousarray(pred[:, :, OUT:])
    return means, logvars


_NC_CACHE = {}


def _get_nc(compute, inner_reps=1, **kw):
    key = (compute, inner_reps, tuple(sorted(kw.items())))
    if key not in _NC_CACHE:
        _NC_CACHE[key] = build_nc(compute, inner_reps, **kw)
    return _NC_CACHE[key]


def kernel(x, W1, b1, W2, b2, W3, b3):
    compute = COMPUTE_DTYPE
    nc = _get_nc(compute, 1, **BEST_KW)
    in_maps = prep_inputs(
        x, W1, b1, W2, b2, W3, b3, compute, l1pack=BEST_KW.get("l1pack", False)
    )
    res = run_bass_kernel_spmd(nc, in_maps, core_ids=list(range(NCORES)))
    return assemble(res.results)
